# revision 9
# baseline (speedup 1.0000x reference)
"""CGNN message-passing kernel for Trainium2, 8 NeuronCores.

Strategy:
  - Algebraic reduction: the edge attention gate depends only on the SOURCE
    node, so attention collapses to a per-node scalar alpha. The whole edge
    computation becomes  aggr = dinv ⊙ (A @ (dinv ⊙ (alpha*xn + (1-alpha)*xa)))
    with A the (multi)adjacency + self loops.
  - Node phase (data-parallel over 8 cores): each core computes its shard of
    the per-node message table m' = dinv*(alpha*xn + (1-alpha)*xa); AllGather
    replicates the full [NP, 128] table to every core.
  - Edge phase (target-sharded): edges sorted by target on host; each core
    owns a contiguous range of target tiles. Per 128-target tile: one
    multi-index indirect DMA gathers all source rows; one-hot matrices built
    from (col % 128) turn scatter-add into PSUM-accumulated matmuls; the
    update/classifier layers fuse in feature-major orientation.

Host work is limited to index preprocessing (sort/bincount/layout) and
shard/unshard data movement; all FLOPs run on device.
"""
import numpy as np

N_CORES = 8
P = 128
IN_DIM = 256
HID = 128
HALF = 64
OUT_DIM = 2
LRELU_SLOPE = 0.01
SLAB_TILES = 4          # node-phase tiles per slab (nn <= 512)


def _host_plan(x, edge_index):
    """Index preprocessing + data layout. Returns dict of np arrays + meta."""
    n = x.shape[0]
    NP = ((n + 1023) // 1024) * 1024          # divisible by 8*128
    nsh = NP // N_CORES                        # nodes per core
    t_c = nsh // P                             # target tiles per core
    ntiles = NP // P

    ei = np.asarray(edge_index)
    row = ei[0].astype(np.int64)
    col = ei[1].astype(np.int64)
    loops = np.arange(n, dtype=np.int64)
    row_a = np.concatenate([row, loops])
    col_a = np.concatenate([col, loops])

    deg = np.bincount(col_a, minlength=NP).astype(np.float32)
    deg[n:] = 1.0                              # pad nodes: keep m' finite

    order = np.argsort(col_a, kind="stable")
    rs = row_a[order].astype(np.int32)
    cs = col_a[order]

    h0 = NP // 2
    assert h0 <= 32767, "table half exceeds int16 index range"
    # order edges by (tile, half) so each tile's lo-half edges precede hi-half
    half_e = (rs >= h0).astype(np.int64)
    key = (cs // P) * 2 + half_e
    order2 = np.argsort(key, kind="stable")
    rs = rs[order2]
    cs = cs[order2]
    key = key[order2]

    gb = np.searchsorted(key, np.arange(0, 2 * (NP // P) + 1))  # group bounds
    glo = gb[0:-1:2]
    ghi = gb[1::2]
    gend = gb[2::2]
    n_lo = ghi - glo                           # per tile lo-edge counts
    n_hi = gend - ghi
    kl_j = -(-n_lo // P)                       # lo chunks per tile
    kh_j = -(-n_hi // P)
    t_slots = ntiles // N_CORES                # == t_c
    KL = np.maximum(1, kl_j.reshape(N_CORES, t_slots).max(0))   # per-slot
    KH = np.maximum(1, kh_j.reshape(N_CORES, t_slots).max(0))
    kmax_tot = int((KL + KH).max())
    kmax8 = int(max(KL.max(), KH.max())) * 8

    # local int16 indices + colmod per (tile, group-chunk-slot)
    idx_lo = np.zeros((ntiles, P, int(KL.max())), np.int16)
    idx_hi = np.zeros((ntiles, P, int(KH.max())), np.int16)
    cm_all = np.full((ntiles, P, kmax_tot), 999.0, np.float32)
    m = len(cs)
    j_e = (cs // P).astype(np.int64)
    is_hi = rs >= h0
    epos = np.arange(m, dtype=np.int64)
    epos = np.where(is_hi, epos - ghi[j_e], epos - glo[j_e])
    c_e = epos // P
    p_e = epos % P
    lo_m = ~is_hi
    idx_lo[j_e[lo_m], p_e[lo_m], c_e[lo_m]] = rs[lo_m].astype(np.int16)
    idx_hi[j_e[is_hi], p_e[is_hi], c_e[is_hi]] = (rs[is_hi] - h0).astype(np.int16)
    cm_e = (cs - j_e * P).astype(np.float32)
    slot_e = j_e % t_slots
    c_cm = np.where(is_hi, KL[slot_e] + c_e, c_e)
    cm_all[j_e, p_e, c_cm] = cm_e

    # wrap + replicate indices for the 8 gpsimd cores: [P, K*8] int16 where
    # block [16g:16g+16, c*8:(c+1)*8] holds chunk c's idxs transposed-wrapped
    def wrap_rep(arr):      # [ntiles, P, K] -> [ntiles, P, K*8]
        nt, _, k = arr.shape
        flat = arr.transpose(0, 2, 1).reshape(nt, k * P)       # chunk-major
        blk = flat.reshape(nt, k * 8, 16).transpose(0, 2, 1)   # [nt, 16, k*8]
        out = np.repeat(blk, 8, axis=0).reshape(nt, 8 * 16, k * 8)
        return np.ascontiguousarray(out)

    idx_cat = np.zeros((ntiles, P, kmax8 * 2), np.int16)
    wlo = wrap_rep(idx_lo)
    whi = wrap_rep(idx_hi)
    idx_cat[:, :, :wlo.shape[2]] = wlo
    idx_cat[:, :, kmax8:kmax8 + whi.shape[2]] = whi

    x_t = np.zeros((IN_DIM, NP), np.float32)
    x_t[:, :n] = np.asarray(x, np.float32).T

    iota = np.tile(np.arange(P, dtype=np.float32), (P, 1))

    return dict(NP=NP, NSH=nsh, T_C=t_c, H0=h0, KL=KL, KH=KH,
                KMAX8=kmax8, KMAX_TOT=kmax_tot,
                idx_all=idx_cat, cm_all=cm_all, deg=deg, x_t=x_t, iota=iota)


def _build_program(meta, with_collective=True, act_lrelu=True):
    import concourse.bass as bass
    import concourse.bacc as bacc
    import concourse.mybir as mybir
    import concourse.tile as tile
    from concourse.masks import make_identity

    f32 = mybir.dt.float32
    i16 = mybir.dt.int16
    NSH, T_C, NP, H0 = meta["NSH"], meta["T_C"], meta["NP"], meta["H0"]
    KL, KH = meta["KL"], meta["KH"]
    KMAX8, KMAX_TOT = meta["KMAX8"], meta["KMAX_TOT"]
    AF = mybir.ActivationFunctionType
    Alu = mybir.AluOpType

    nc = bacc.Bacc("TRN2", target_bir_lowering=False, debug=False)
    table = nc.dram_tensor("cc_table", [NP, HID], f32, addr_space="Shared")

    def emit_lrelu(out_ap, psum_ap, bias_ap, tmp):
        # out = leaky_relu(psum + bias); ACT Lrelu on HW, decomposition in sim
        if act_lrelu:
            nc.scalar.activation(out_ap, psum_ap, AF.Lrelu, bias=bias_ap,
                                 alpha=LRELU_SLOPE)
        else:
            nc.scalar.activation(out_ap, psum_ap, AF.Identity, bias=bias_ap)
            nc.vector.tensor_scalar(out=tmp, in0=out_ap, scalar1=LRELU_SLOPE,
                                    scalar2=None, op0=Alu.mult)
            nc.vector.tensor_tensor(out=out_ap, in0=out_ap, in1=tmp,
                                    op=Alu.max)

    # ---- external inputs (per-core shards unless noted)
    d_xt = nc.dram_tensor("x_t", [IN_DIM, NSH], f32, kind="ExternalInput")
    d_deg = nc.dram_tensor("deg", [NSH], f32, kind="ExternalInput")
    d_idx = nc.dram_tensor("idx", [T_C, P, KMAX8 * 2], i16, kind="ExternalInput")
    d_cm = nc.dram_tensor("cm", [T_C, P, KMAX_TOT], f32, kind="ExternalInput")
    d_iota = nc.dram_tensor("iota", [P, P], f32, kind="ExternalInput")
    d_w_in = nc.dram_tensor("W_in", [IN_DIM, HID], f32, kind="ExternalInput")
    d_b_in = nc.dram_tensor("b_in", [HID, 1], f32, kind="ExternalInput")
    d_w_nor = nc.dram_tensor("W_nor", [HALF, HID], f32, kind="ExternalInput")
    d_b_nor = nc.dram_tensor("b_nor", [HID, 1], f32, kind="ExternalInput")
    d_w_ab = nc.dram_tensor("W_abnor", [HALF, HID], f32, kind="ExternalInput")
    d_b_ab = nc.dram_tensor("b_abnor", [HID, 1], f32, kind="ExternalInput")
    d_w_att = nc.dram_tensor("W_att", [HID, HID], f32, kind="ExternalInput")
    d_b_att = nc.dram_tensor("b_att", [HID, 1], f32, kind="ExternalInput")
    d_v_att = nc.dram_tensor("v_att", [HID, 1], f32, kind="ExternalInput")
    d_w_upd = nc.dram_tensor("W_upd", [HID, HID], f32, kind="ExternalInput")
    d_b_upd = nc.dram_tensor("b_upd", [HID, 1], f32, kind="ExternalInput")
    d_w_cls = nc.dram_tensor("W_cls", [HID, OUT_DIM], f32, kind="ExternalInput")
    d_b_cls = nc.dram_tensor("b_cls", [OUT_DIM, 1], f32, kind="ExternalInput")
    d_out = nc.dram_tensor("outp", [OUT_DIM, NSH], f32, kind="ExternalOutput")

    with tile.TileContext(nc) as tc:
        with (
            tc.tile_pool(name="const", bufs=1) as cpool,
            tc.tile_pool(name="sbuf", bufs=2) as pool,
            tc.tile_pool(name="sb3", bufs=3) as pool3,
            tc.tile_pool(name="dram", bufs=1, space="DRAM") as dpool,
        ):
            # ---------- persistent constants ----------
            w_in_a = cpool.tile([P, HID], f32)
            w_in_b = cpool.tile([P, HID], f32)
            nc.sync.dma_start(w_in_a[:], d_w_in[:P, :])
            nc.sync.dma_start(w_in_b[:], d_w_in[P:, :])
            w_nor = cpool.tile([P, HID], f32)     # zero-extended K=128
            w_ab = cpool.tile([P, HID], f32)
            nc.vector.memset(w_nor[:], 0.0)
            nc.vector.memset(w_ab[:], 0.0)
            nc.sync.dma_start(w_nor[:HALF, :], d_w_nor[:])
            nc.sync.dma_start(w_ab[HALF:, :], d_w_ab[:])
            w_att = cpool.tile([P, HID], f32)
            nc.sync.dma_start(w_att[:], d_w_att[:])
            v_att = cpool.tile([P, 1], f32)
            nc.sync.dma_start(v_att[:], d_v_att[:])
            w_upd = cpool.tile([P, HID], f32)
            nc.sync.dma_start(w_upd[:], d_w_upd[:])
            w_cls = cpool.tile([P, OUT_DIM], f32)
            nc.sync.dma_start(w_cls[:], d_w_cls[:])
            b_in = cpool.tile([P, 1], f32)
            nc.sync.dma_start(b_in[:], d_b_in[:])
            b_nor = cpool.tile([P, 1], f32)
            nc.sync.dma_start(b_nor[:], d_b_nor[:])
            b_ab = cpool.tile([P, 1], f32)
            nc.sync.dma_start(b_ab[:], d_b_ab[:])
            b_att = cpool.tile([P, 1], f32)
            nc.sync.dma_start(b_att[:], d_b_att[:])
            b_upd = cpool.tile([P, 1], f32)
            nc.sync.dma_start(b_upd[:], d_b_upd[:])
            b_cls = cpool.tile([OUT_DIM, 1], f32)
            nc.sync.dma_start(b_cls[:], d_b_cls[:])
            iota_t = cpool.tile([P, P], f32)
            nc.sync.dma_start(iota_t[:], d_iota[:])
            ones_r = cpool.tile([1, P], f32)
            nc.vector.memset(ones_r[:], 1.0)
            ident = cpool.tile([P, P], f32)
            make_identity(nc, ident[:])

            # dinv row for this core's nodes: 1/sqrt(deg)
            dinvr = cpool.tile([1, NSH], f32)
            nc.sync.dma_start(dinvr[:], d_deg[:][None, :])
            nc.scalar.activation(dinvr[:], dinvr[:], AF.Sqrt)
            nc.vector.reciprocal(dinvr[:], dinvr[:])

            # message table (gather source) + local shard
            shard = dpool.tile([NSH, HID], f32)

            # ---------- node phase (this core's NSH nodes) ----------
            npsum = tc.tile_pool(name="npsum", bufs=2, space="PSUM")
            pp1 = pp2 = npsum.__enter__()
            slabs = []
            t0 = 0
            while t0 < T_C:
                nt = min(SLAB_TILES, T_C - t0)
                slabs.append((t0, nt))
                t0 += nt
            for (s0, nt) in slabs:
                nn = nt * P
                nb = s0 * P
                xta = pool.tile([P, 512], f32, tag="xta")
                xtb = pool.tile([P, 512], f32, tag="xtb")
                nc.sync.dma_start(xta[:, :nn], d_xt[:P, nb:nb + nn])
                nc.sync.dma_start(xtb[:, :nn], d_xt[P:, nb:nb + nn])
                ph = pp1.tile([P, 512], f32, tag="ph")
                nc.tensor.matmul(ph[:, :nn], w_in_a[:], xta[:, :nn],
                                 start=True, stop=False)
                nc.tensor.matmul(ph[:, :nn], w_in_b[:], xtb[:, :nn],
                                 start=False, stop=True)
                h = pool.tile([P, 512], f32, tag="h")
                ltmp = pool.tile([P, 512], f32, tag="ltmp")
                emit_lrelu(h[:, :nn], ph[:, :nn], b_in[:], ltmp[:, :nn])
                pn = pp1.tile([P, 512], f32, tag="pnpa")
                pa = pp1.tile([P, 512], f32, tag="pnpa")
                nc.tensor.matmul(pn[:, :nn], w_nor[:], h[:, :nn],
                                 start=True, stop=True)
                nc.tensor.matmul(pa[:, :nn], w_ab[:], h[:, :nn],
                                 start=True, stop=True)
                xn = pool.tile([P, 512], f32, tag="xn")
                xa = pool.tile([P, 512], f32, tag="xa")
                nc.scalar.activation(xn[:, :nn], pn[:, :nn], AF.Identity,
                                     bias=b_nor[:])
                nc.scalar.activation(xa[:, :nn], pa[:, :nn], AF.Identity,
                                     bias=b_ab[:])
                s_t = pool.tile([P, 512], f32, tag="s")
                nc.vector.tensor_add(s_t[:, :nn], xn[:, :nn], xa[:, :nn])
                patt = pp1.tile([P, 512], f32, tag="ph")
                nc.tensor.matmul(patt[:, :nn], w_att[:], s_t[:, :nn],
                                 start=True, stop=True)
                hatt = pool.tile([P, 512], f32, tag="hatt")
                nc.scalar.activation(hatt[:, :nn], patt[:, :nn], AF.Tanh,
                                     bias=b_att[:])
                pal = pp1.tile([1, 512], f32, tag="sm")
                nc.tensor.matmul(pal[:, :nn], v_att[:], hatt[:, :nn],
                                 start=True, stop=True)
                # a2 = sigmoid(alpha_pre) * dinv ; da = dinv - a2   (rows)
                a2r = pool.tile([1, 512], f32, tag="a2r")
                nc.scalar.activation(a2r[:, :nn], pal[:, :nn], AF.Sigmoid)
                nc.vector.tensor_mul(a2r[:, :nn], a2r[:, :nn],
                                     dinvr[:, nb:nb + nn])
                dar = pool.tile([1, 512], f32, tag="dar")
                nc.vector.tensor_sub(dar[:, :nn], dinvr[:, nb:nb + nn],
                                     a2r[:, :nn])
                for j in range(nt):
                    jn = j * P
                    pb = pp2.tile([P, 256], f32, tag="sm")
                    nc.tensor.matmul(pb[:, :P], ones_r[:],
                                     a2r[:, jn:jn + P], start=True, stop=True)
                    nc.tensor.matmul(pb[:, P:], ones_r[:],
                                     dar[:, jn:jn + P], start=True, stop=True)
                    bc = pool.tile([P, 256], f32, tag="bc")
                    nc.vector.tensor_copy(bc[:], pb[:])
                    t2 = pool.tile([P, P], f32, tag="t2")
                    nc.vector.tensor_mul(t2[:], xn[:, jn:jn + P], bc[:, :P])
                    t3 = pool.tile([P, P], f32, tag="t3")
                    nc.vector.tensor_mul(t3[:], xa[:, jn:jn + P], bc[:, P:])
                    mt = pool.tile([P, P], f32, tag="mt")
                    nc.vector.tensor_add(mt[:], t2[:], t3[:])
                    ptr = pp2.tile([P, P], f32, tag="sm")
                    nc.tensor.transpose(ptr[:], mt[:], ident[:])
                    mrow = pool.tile([P, P], f32, tag="mrow")
                    nc.scalar.activation(mrow[:], ptr[:], AF.Identity)
                    nc.sync.dma_start(
                        shard[(s0 + j) * P:(s0 + j + 1) * P, :], mrow[:])

            npsum.__exit__(None, None, None)

            # ---------- replicate table ----------
            if with_collective:
                nc.gpsimd.collective_compute(
                    "AllGather",
                    mybir.AluOpType.bypass,
                    replica_groups=[list(range(N_CORES))],
                    ins=[shard.opt()],
                    outs=[table[:]],
                )
            else:
                # cost-model / single-core mode: fake it with a local copy
                nc.sync.dma_start(table[:NSH, :], shard[:])

            # ---------- edge phase (this core's T_C target tiles) ----------
            epsum = tc.tile_pool(name="epsum", bufs=2, space="PSUM")
            pp2 = epsum.__enter__()
            for j in range(T_C):
                kl, kh = int(KL[j]), int(KH[j])
                kt = kl + kh
                it = pool3.tile([P, KMAX8 * 2], i16, tag="it")
                nc.sync.dma_start(it[:], d_idx[j])
                cmt = pool3.tile([P, KMAX_TOT], f32, tag="cmt")
                nc.sync.dma_start(cmt[:], d_cm[j])
                g = pool.tile([P, KMAX_TOT * HID], f32, tag="g")
                # SWDGE ring caps one gather at 1024 descriptors -> <=8 chunks
                segs = []
                for c0 in range(0, kl, 8):
                    segs.append((c0, min(8, kl - c0), 0, c0 * 8))
                for c0 in range(0, kh, 8):
                    segs.append((kl + c0, min(8, kh - c0), H0,
                                 KMAX8 + c0 * 8))
                for (dst_c, nseg, base, io) in segs:
                    nc.gpsimd.dma_gather(
                        out_ap=g[:, dst_c * HID:(dst_c + nseg) * HID]
                        .rearrange("p (c f) -> p c f", f=HID),
                        in_ap=table[base:base + H0, :],
                        idxs_ap=it[:, io:io + nseg * 8],
                        num_idxs=nseg * P,
                        num_idxs_reg=nseg * P,
                        elem_size=HID,
                    )
                pagg = pp2.tile([P, P], f32, tag="pagg")
                for c in range(kt):
                    s_oh = pool3.tile([P, P], f32, tag="soh")
                    nc.vector.tensor_scalar(
                        out=s_oh[:], in0=iota_t[:], scalar1=cmt[:, c:c + 1],
                        scalar2=None, op0=Alu.is_equal)
                    nc.tensor.matmul(pagg[:], g[:, c * HID:(c + 1) * HID],
                                     s_oh[:], start=(c == 0),
                                     stop=(c == kt - 1))
                jn = j * P
                pdb = pp2.tile([P, P], f32, tag="emisc")
                nc.tensor.matmul(pdb[:], ones_r[:], dinvr[:, jn:jn + P],
                                 start=True, stop=True)
                dbc = pool.tile([P, P], f32, tag="dbc")
                nc.vector.tensor_copy(dbc[:], pdb[:])
                m2 = pool.tile([P, P], f32, tag="m2")
                nc.vector.tensor_mul(m2[:], pagg[:], dbc[:])
                pu = pp2.tile([P, P], f32, tag="emisc")
                nc.tensor.matmul(pu[:], w_upd[:], m2[:], start=True, stop=True)
                lu = pool.tile([P, P], f32, tag="lu")
                ltm2 = pool.tile([P, P], f32, tag="ltm2")
                emit_lrelu(lu[:], pu[:], b_upd[:], ltm2[:])
                po = pp2.tile([OUT_DIM, P], f32, tag="emisc")
                nc.tensor.matmul(po[:], w_cls[:], lu[:], start=True, stop=True)
                ot = pool.tile([OUT_DIM, P], f32, tag="ot")
                nc.scalar.activation(ot[:], po[:], AF.Identity, bias=b_cls[:])
                nc.sync.dma_start(d_out[:, jn:jn + P], ot[:])
            epsum.__exit__(None, None, None)

    nc.compile()
    return nc


def _run_spmd_presharded(nc, in_maps, n_cores=N_CORES):
    """Run a compiled Bass program on n_cores via PJRT with host-side
    pre-sharded inputs (avoids XLA reshard programs on big arrays)."""
    import jax
    import concourse.mybir as mybir
    from concourse import bass2jax
    from jax.sharding import Mesh, PartitionSpec, NamedSharding
    from jax.experimental.shard_map import shard_map

    bass2jax.install_neuronx_cc_hook()
    partition_name = nc.partition_id_tensor.name if nc.partition_id_tensor else None
    in_names, out_names, out_avals, zero_outs = [], [], [], []
    for alloc in nc.m.functions[0].allocations:
        if not isinstance(alloc, mybir.MemoryLocationSet):
            continue
        name = alloc.memorylocations[0].name
        if alloc.kind == "ExternalInput":
            if name != partition_name:
                in_names.append(name)
        elif alloc.kind == "ExternalOutput":
            out_names.append(name)
            shape = tuple(alloc.tensor_shape)
            dtype = mybir.dt.np(alloc.dtype)
            out_avals.append(jax.core.ShapedArray(shape, dtype))
            zero_outs.append(np.zeros(shape, dtype))
    n_params = len(in_names)
    in_names_all = list(in_names) + out_names
    if partition_name is not None:
        in_names_all.append(partition_name)

    def _body(*args):
        operands = list(args)
        if partition_name is not None:
            operands.append(bass2jax.partition_id_tensor())
        outs = bass2jax._bass_exec_p.bind(
            *operands,
            out_avals=tuple(out_avals),
            in_names=tuple(in_names_all),
            out_names=tuple(out_names),
            lowering_input_output_aliases=(),
            sim_require_finite=True,
            sim_require_nnan=True,
            nc=nc,
        )
        return tuple(outs)

    devices = jax.devices()[:n_cores]
    mesh = Mesh(np.asarray(devices), ("core",))
    spec = PartitionSpec("core")
    n_outs = len(out_avals)
    sharded = jax.jit(
        shard_map(_body, mesh=mesh, in_specs=(spec,) * (n_params + n_outs),
                  out_specs=(spec,) * n_outs, check_rep=False),
        keep_unused=True,
    )
    sh = NamedSharding(mesh, spec)

    def put(per_core_arrays):
        a0 = np.asarray(per_core_arrays[0])
        gshape = (n_cores * a0.shape[0],) + a0.shape[1:]
        shards = [jax.device_put(np.ascontiguousarray(per_core_arrays[c]),
                                 devices[c]) for c in range(n_cores)]
        return jax.make_array_from_single_device_arrays(gshape, sh, shards)

    args = [put([m[name] for m in in_maps]) for name in in_names]
    args += [put([z] * n_cores) for z in zero_outs]
    out_arrs = sharded(*args)
    jax.block_until_ready(out_arrs)
    return [
        {name: np.asarray(out_arrs[i]).reshape(n_cores, *out_avals[i].shape)[c]
         for i, name in enumerate(out_names)}
        for c in range(n_cores)
    ]


def kernel(x, edge_index, W_in, b_in, W_nor, b_nor, W_abnor, b_abnor,
           W_att, b_att, v_att, W_upd, b_upd, W_cls, b_cls):
    x = np.asarray(x, np.float32)
    n = x.shape[0]
    meta = _host_plan(x, edge_index)
    NSH, T_C = meta["NSH"], meta["T_C"]
    nc = _build_program(meta, with_collective=True)

    shared = {
        "iota": meta["iota"],
        "W_in": np.asarray(W_in, np.float32),
        "b_in": np.asarray(b_in, np.float32).reshape(HID, 1),
        "W_nor": np.asarray(W_nor, np.float32),
        "b_nor": np.asarray(b_nor, np.float32).reshape(HID, 1),
        "W_abnor": np.asarray(W_abnor, np.float32),
        "b_abnor": np.asarray(b_abnor, np.float32).reshape(HID, 1),
        "W_att": np.asarray(W_att, np.float32),
        "b_att": np.asarray(b_att, np.float32).reshape(HID, 1),
        "v_att": np.asarray(v_att, np.float32).reshape(HID, 1),
        "W_upd": np.asarray(W_upd, np.float32),
        "b_upd": np.asarray(b_upd, np.float32).reshape(HID, 1),
        "W_cls": np.asarray(W_cls, np.float32),
        "b_cls": np.asarray(b_cls, np.float32).reshape(OUT_DIM, 1),
    }
    idx_c = meta["idx_all"].reshape(N_CORES, T_C, P, meta["KMAX8"] * 2)
    cm_c = meta["cm_all"].reshape(N_CORES, T_C, P, meta["KMAX_TOT"])
    in_maps = []
    for c in range(N_CORES):
        in_maps.append({
            **shared,
            "x_t": np.ascontiguousarray(meta["x_t"][:, c * NSH:(c + 1) * NSH]),
            "deg": meta["deg"][c * NSH:(c + 1) * NSH],
            "idx": idx_c[c],
            "cm": cm_c[c],
        })

    results = _run_spmd_presharded(nc, in_maps)
    out_t = np.concatenate([results[c]["outp"] for c in range(N_CORES)],
                           axis=1)        # [2, NP]
    return np.ascontiguousarray(out_t[:, :n].T)
